# revision 13
# baseline (speedup 1.0000x reference)
"""MoE FFN (BertGeneration-style) on 8 TRN2 NeuronCores, expert-parallel.

Problem: 8192 tokens, expert = task_id % 8, per-expert FFN
(768 -> 3072 gelu -> 768) + residual + per-expert LayerNorm.

Strategy: routing (dispatch/combine) is a host-side permutation; each of the
8 cores runs one expert's FFN over its 1024-token block.  On-chip:
  phase 1:  hT[i, m] = gelu(sum_k W1[k, i] * xT[k, m] + b1[i])   (h transposed)
  phase 2:  y[m, h]  = sum_i hT[i, m] * W2[i, h];  z = y + (x + b2);
            LayerNorm(z) along h.
h stays transposed in SBUF so GEMM1's output is directly GEMM2's stationary
operand.  Matmuls run in float32r (full-rate fp32 streaming mode).
"""

import sys

if "/opt/trn_rl_repo" not in sys.path:
    sys.path.insert(0, "/opt/trn_rl_repo")

import numpy as np

def _install_axon_hooks_shim():
    """Provide antenv.axon_hooks (NTFF profiling hook) when the image's
    antenv lacks it — a thin ctypes wrapper over libaxon_pjrt.so, matching
    trn_agent_boot.trn_boot._ntff_profile_via_ctypes.  Only exercised when
    profiling is requested (BASS_TRACE); harmless otherwise."""
    import contextlib
    import ctypes
    import types

    try:
        import antenv.axon_hooks  # noqa: F401
        return
    except ImportError:
        pass
    try:
        import antenv
    except ImportError:
        return

    mod = types.ModuleType("antenv.axon_hooks")
    _state = {"hook": None, "init": False}

    def set_axon_ntff_profile_hook(h):
        _state["hook"] = h
        _state["init"] = True

    def get_axon_ntff_profile_hook():
        if _state["init"]:
            return _state["hook"]
        _state["init"] = True
        try:
            lib = ctypes.CDLL("/opt/axon/libaxon_pjrt.so")
        except OSError:
            return None
        if not hasattr(lib, "axon_start_nrt_profile"):
            return None
        lib.axon_start_nrt_profile.argtypes = [
            ctypes.POINTER(ctypes.c_int64), ctypes.c_size_t]
        lib.axon_start_nrt_profile.restype = ctypes.c_int64
        lib.axon_stop_nrt_profile.argtypes = [ctypes.c_char_p]
        lib.axon_stop_nrt_profile.restype = ctypes.c_int64

        @contextlib.contextmanager
        def _hook(output_dir, device_ids):
            import jax
            jax.devices()
            if device_ids:
                ids = (ctypes.c_int64 * len(device_ids))(*device_ids)
                rc = lib.axon_start_nrt_profile(ids, len(device_ids))
            else:
                rc = lib.axon_start_nrt_profile(None, 0)
            if rc != 0:
                raise RuntimeError(f"axon_start_nrt_profile rc={rc}")
            try:
                yield
            finally:
                n = lib.axon_stop_nrt_profile(str(output_dir).encode())
                print(f"profile: {n} file(s) written to {output_dir}")

        _state["hook"] = _hook
        return _hook

    mod.set_axon_ntff_profile_hook = set_axon_ntff_profile_hook
    mod.get_axon_ntff_profile_hook = get_axon_ntff_profile_hook
    sys.modules["antenv.axon_hooks"] = mod
    antenv.axon_hooks = mod


_install_axon_hooks_shim()

E = 8
N = 8192
H = 768
I = 3072
C = N // E        # 1024 tokens per expert/core
KT = H // 128     # 6   k-tiles (hidden dim)
IT = I // 128     # 24  i-tiles (intermediate dim)
MT = C // 128     # 8   m-tiles (token dim per core)
EPS = 1e-12
W2E = 16          # W2 k-tiles prefetched during phase 1 (rest after xT freed)

_CACHE = {}


def _build_nc(act_name="Gelu"):
    from contextlib import ExitStack

    import concourse.tile as tile
    from concourse import bacc, mybir

    f32 = mybir.dt.float32
    f32r = mybir.dt.float32r
    AF = mybir.ActivationFunctionType
    act_fn = getattr(AF, act_name)
    ALU = mybir.AluOpType

    nc = bacc.Bacc("TRN2", target_bir_lowering=False, debug=False, num_devices=8)

    # matmul operands travel as float32r (same 32-bit payload; PE streams it
    # at full rate) — declared f32r end-to-end so the BIR verifier sees
    # rounded producers for every fp32r matmult input
    xT3 = nc.dram_tensor("xT3", [128, KT, C], f32r, kind="ExternalInput").ap()
    xn = nc.dram_tensor("xn", [128, MT, H], f32, kind="ExternalInput").ap()
    w1 = nc.dram_tensor("w1", [128, IT, KT, 128], f32r, kind="ExternalInput").ap()
    w2 = nc.dram_tensor("w2", [128, IT, H], f32r, kind="ExternalInput").ap()
    b1t = nc.dram_tensor("b1t", [128, IT], f32, kind="ExternalInput").ap()
    out = nc.dram_tensor("out", [128, MT, H], f32, kind="ExternalOutput").ap()

    with ExitStack() as ctx:
        tc = ctx.enter_context(tile.TileContext(nc))
        persist = ctx.enter_context(tc.tile_pool(name="persist", bufs=1))
        psum1 = ctx.enter_context(tc.tile_pool(name="psum1", bufs=2, space="PSUM"))
        psum2 = ctx.enter_context(tc.tile_pool(name="psum2", bufs=2, space="PSUM"))

        hT = persist.tile([128, IT, C], f32r, name="hT")
        w2e = persist.tile([128, W2E, H], f32r, name="w2e")
        b1s = persist.tile([128, IT], f32, name="b1s")
        epsT = persist.tile([128, 1], f32, name="epsT")

        nc.vector.memset(epsT, EPS)

        # ---- phase 1: hT = gelu(W1.T @ xT + b1) ----
        with tc.tile_pool(name="ph1", bufs=1) as ph1, \
             tc.tile_pool(name="w1s", bufs=3) as w1pool:
            # DMA order: first W1 i-tile, then xT k-chunks, so the PE can
            # start on (it=0, kt=0) as soon as ~900KB has landed instead of
            # waiting for the full 3MB xT transfer.
            w1t0 = w1pool.tile([128, KT, 128], f32r, name="w1t", tag="w1t")
            nc.sync.dma_start(out=w1t0, in_=w1[:, 0])
            xT3s = ph1.tile([128, KT, C], f32r, name="xT3s")
            for kt in range(KT):
                nc.sync.dma_start(out=xT3s[:, kt], in_=xT3[:, kt])
            nc.sync.dma_start(out=b1s, in_=b1t)
            for it in range(IT):
                if it == 0:
                    w1t = w1t0
                else:
                    w1t = w1pool.tile([128, KT, 128], f32r, name="w1t", tag="w1t")
                    nc.sync.dma_start(out=w1t, in_=w1[:, it])
                ph = psum1.tile([128, C], f32, name="ph", tag="ph")
                for kt in range(KT):
                    lhsT = w1t[:, kt, :]
                    for half in range(2):
                        nc.tensor.matmul(
                            ph[:, half * 512:(half + 1) * 512],
                            lhsT=lhsT,
                            rhs=xT3s[:, kt, half * 512:(half + 1) * 512],
                            start=(kt == 0),
                            stop=(kt == KT - 1),
                        )
                nc.scalar.activation(hT[:, it, :], ph, act_fn, bias=b1s[:, it:it + 1])
                if it < W2E:
                    nc.sync.dma_start(out=w2e[:, it], in_=w2[:, it])

        # ---- phase 2: y = hT.T @ W2; z = y + xn; LayerNorm ----
        with tc.tile_pool(name="late", bufs=1) as late, \
             tc.tile_pool(name="xns", bufs=3) as xpool, \
             tc.tile_pool(name="zs", bufs=3) as zpool, \
             tc.tile_pool(name="small", bufs=6) as spool:
            w2l = late.tile([128, IT - W2E, H], f32r, name="w2l")
            for j in range(IT - W2E):
                nc.sync.dma_start(out=w2l[:, j], in_=w2[:, W2E + j])
            for mt in range(MT):
                xnt = xpool.tile([128, H], f32, name="xnt", tag="xnt")
                nc.gpsimd.dma_start(out=xnt, in_=xn[:, mt])
                py = psum2.tile([128, C], f32, name="py", tag="py")
                for it in range(IT):
                    w2src = w2e[:, it] if it < W2E else w2l[:, it - W2E]
                    lhsT = hT[:, it, mt * 128:(mt + 1) * 128]
                    nc.tensor.matmul(
                        py[:, 0:512], lhsT=lhsT, rhs=w2src[:, 0:512],
                        start=(it == 0), stop=(it == IT - 1))
                    nc.tensor.matmul(
                        py[:, 512:768], lhsT=lhsT, rhs=w2src[:, 512:768],
                        start=(it == 0), stop=(it == IT - 1))
                z = zpool.tile([128, H], f32, name="z", tag="z")
                nc.vector.tensor_add(z, py[:, 0:H], xnt)
                stats = spool.tile([128, 3, 6], f32, name="stats", tag="stats")
                for sg in range(3):
                    nc.vector.bn_stats(stats[:, sg], z[:, sg * 256:(sg + 1) * 256])
                mv = spool.tile([128, 2], f32, name="mv", tag="mv")
                nc.vector.bn_aggr(mv, stats)
                rstd = spool.tile([128, 1], f32, name="rstd", tag="rstd")
                nc.scalar.activation(rstd, mv[:, 1:2], AF.Sqrt, bias=epsT)
                nc.vector.reciprocal(out=rstd, in_=rstd)
                nc.vector.tensor_scalar(
                    out=z, in0=z, scalar1=mv[:, 0:1], scalar2=rstd,
                    op0=ALU.subtract, op1=ALU.mult)
                nc.gpsimd.dma_start(out=out[:, mt], in_=z)

    nc.compile()
    return nc


def _get_nc(act_name="Gelu"):
    key = ("nc", act_name)
    if key not in _CACHE:
        _CACHE[key] = _build_nc(act_name)
    return _CACHE[key]


def _shard_inputs(x, task_ids, W1, b1, W2, b2):
    """Host-side dispatch: stable-sort tokens by expert id, chunk into E
    equal capacity-C blocks (exactly the reference's xs = x[order].reshape)."""
    expert = (task_ids.astype(np.int64) % E).astype(np.int32)
    order = np.argsort(expert, kind="stable")
    xs = x[order]
    in_maps = []
    for e in range(E):
        xe = xs[e * C:(e + 1) * C]                       # [C, H]
        xT3 = xe.T.reshape(KT, 128, C).transpose(1, 0, 2)
        xn = (xe + b2[e][None, :]).reshape(MT, 128, H).transpose(1, 0, 2)
        w1 = W1[e].reshape(KT, 128, IT, 128).transpose(1, 2, 0, 3)
        w2 = W2[e].reshape(IT, 128, H).transpose(1, 0, 2)
        b1t = b1[e].reshape(IT, 128).T
        in_maps.append({
            "xT3": np.ascontiguousarray(xT3, dtype=np.float32),
            "xn": np.ascontiguousarray(xn, dtype=np.float32),
            "w1": np.ascontiguousarray(w1, dtype=np.float32),
            "w2": np.ascontiguousarray(w2, dtype=np.float32),
            "b1t": np.ascontiguousarray(b1t, dtype=np.float32),
        })
    return in_maps, order


def kernel(x, task_ids, W1, b1, W2, b2, gamma, beta):
    from concourse import bass_utils

    x = np.asarray(x, dtype=np.float32)
    task_ids = np.asarray(task_ids)
    W1 = np.asarray(W1, dtype=np.float32)
    b1 = np.asarray(b1, dtype=np.float32)
    W2 = np.asarray(W2, dtype=np.float32)
    b2 = np.asarray(b2, dtype=np.float32)
    gamma = np.asarray(gamma, dtype=np.float32)
    beta = np.asarray(beta, dtype=np.float32)

    in_maps, order = _shard_inputs(x, task_ids, W1, b1, W2, b2)
    nc = _get_nc()
    res = bass_utils.run_bass_kernel_spmd(nc, in_maps, core_ids=list(range(E)))
    _CACHE["last_results"] = res

    z = np.concatenate(
        [res.results[e]["out"].transpose(1, 0, 2).reshape(C, H) for e in range(E)],
        axis=0)
    # per-expert gamma/beta (identity for this problem's inputs; applied on
    # host only when nontrivial, matching the reference's z*gamma + beta)
    if not (np.all(gamma == 1.0) and np.all(beta == 0.0)):
        expert_sorted = (task_ids.astype(np.int64)[order] % E).astype(np.int32)
        blk = np.repeat(np.arange(E), C)  # reference uses capacity blocks
        del expert_sorted
        z = z * gamma[blk] + beta[blk]
    out = np.empty((N, H), dtype=np.float32)
    out[order] = z
    return out
